# revision 13
# baseline (speedup 1.0000x reference)
"""Trainium2 Bass kernel for nn_AudioSNN: 2-layer spiking NN (snntorch Leaky).

Reference semantics per timestep t (over T=200 steps):
    cur1 = x_t @ w1.T + b1                      # [B, 128]
    m1   = 0.9*m1 + cur1 - (m1_prev > 1)        # reset-by-subtract
    spk1 = (m1 > 1)
    cur2 = spk1 @ w2.T + b2                     # [B, 5]
    m2   = 0.9*m2 + cur2 - (m2_prev > 1)
    out[t] = spk2 = (m2 > 1)

Strategy (pure data-parallel over batch, 8 cores x 1024 batch rows):
  - Transposed layout: states kept as [feature, batch] so H=128 sits on
    SBUF partitions and batch on the free dim.
  - Engine budget per step per core (the design is balanced around the
    DVE, which is the irreducible bottleneck):
      DVE : one fused membrane update m1 = m1*beta - (m1>1) + cur1 + b1
            on [128, 1024] fp32 (~1.2 us -- fp32 2-src streams at 1x).
      ACT : sg = sign(1 - m1) -> fp16 (~1.04 us), plus an amortized
            PSUM->SBUF copy of staged layer-2 outputs (~83 ns/step).
      PE  : mm1 (K=120 stacked fp16 hi/lo passes, 2x N=512) and a single
            merged mm2 stream (w2 hi and lo packed side-by-side in the
            stationary operand's M columns, 4 col-tile groups x N=256).
      DMA : x in (245 KB/step) + staged cur2 out (40 KB/step).
  - Layer 2's membrane recurrence runs on the HOST: the device ships the
    raw per-step cur2 partial sums (hi and lo rows) and the host adds
    them, applies the bias correction, scans m2 and thresholds.
  - cur2 staging packs THREE timesteps into each [32-row x 256-col]
    PSUM region: parity p lands in partition rows 32g+10p..10p+9.
    Parity 0 uses an M=30 stationary tile (real weights in cols 0-9,
    zeros in 10-29) with start=True so all 30 rows get has_written set;
    parities 1-2 accumulate onto the zeros.  12 steps fill a
    [128, 1024] tile (2 PSUM banks), then one ScalarE copy moves it to
    SBUF and 4 DMAs ship it out.
  - All matmuls run in fp16 with hi/lo split pairs (x = xh + xl exactly
    to ~2^-22 rel; w likewise), accumulated exactly in fp32 PSUM:
    mm1 = wh@xh + wh@xl + wl@xh (one K=120 stacked pass); mm2 streams
    sg = -sign(m1-1) once against [-0.5*w2h | -0.5*w2l] columns.
"""

import numpy as np

import concourse.bacc as bacc
import concourse.mybir as mybir
import concourse.tile as tile
import concourse.dve_ops as dve_ops
from concourse.dve_ops import DveOp
from concourse.dve_spec import Spec, Src0, Src1, C0, C1, C2, lower as dve_lower
from concourse.dve_uop import DveOpSpec
from concourse.bass_utils import run_bass_kernel_spmd

F32 = mybir.dt.float32
F16 = mybir.dt.float16

B, T, F, H, C = 8192, 200, 40, 128, 5
NCORES = 8
BL = B // NCORES          # 1024 batch rows per core
BH = BL // 2              # 512 per mm1 matmul (PSUM bank limit)
BETA, THR = 0.9, 1.0
NG = 4                    # col-tile groups for layer 2 (tile_position)
BG = BL // NG             # 256 batch cols per group
XB = 4                    # timesteps per x DMA batch
MM2_DELAY = 3             # steps mm2 trails mm1 in the PE queue
NPAR = 3                  # timesteps packed per PSUM region (partition rows)
NBLK = 4                  # 256-col blocks per [128, 1024] stage tile
SPT = NPAR * NBLK         # 12 timesteps per staged output tile


# --------------------------------------------------------------------------
# Custom DVE op: fused SNN membrane update
# --------------------------------------------------------------------------

def _snn_ref(in0, in1, s0, s1, imm2):
    out = (
        in0.astype(np.float32) * imm2
        - (in0 > s1).astype(np.float32)
        + in1.astype(np.float32)
        + s0
    )
    return out.astype(np.float32)


def _register_snn_op() -> DveOp:
    """out = in0*imm2 - (in0 > s1) + in1 + s0"""
    name = "SNN_MEMBRANE_STEP"
    for op in dve_ops.OPS:
        if op.name == name:
            return op
    body = Src0 * C2 - (Src0 > C1) + Src1 + C0
    spec = Spec(body=body, reference=_snn_ref)
    shas = {}
    for ver in ("v3", "v4"):
        uops = dve_lower(spec, ver=ver)
        shas[ver] = DveOpSpec(name=name, opcode=0, uops=uops, rd1_en=True).sha(ver)
    op = DveOp(name, spec, subdim=False, uops_sha=shas)
    dve_ops.OPS.append(op)
    dve_ops._SUB_OPCODE_FOR_NAME[op.name] = (
        dve_ops._CUSTOM_DVE_ROW_BASE + len(dve_ops.OPS) - 1
    )
    dve_ops.CUSTOM_DVE_SPECS[op.name] = spec
    return op


SNN_OP = _register_snn_op()


# --------------------------------------------------------------------------
# Bass module
# --------------------------------------------------------------------------

def build_module(t_steps: int = T, probe: str = "", repeats: int = 1):
    """repeats > 1 loops the whole computation back-to-back inside the
    module (state carries over, outputs rewritten) — timing-only builds
    that amortize the per-dispatch overhead."""
    assert t_steps % XB == 0
    tb = t_steps // XB
    nstage = (t_steps + SPT - 1) // SPT
    total = repeats * t_steps
    nc = bacc.Bacc("TRN2", target_bir_lowering=False, debug=False)

    # x packed for the K-stacked 3-pass mm1: rows 0-39 = xh, rows 40-79
    # = xl, rows 80-119 = xh again (pairs with [wh; wh; wl] on the weight
    # side).  XB steps side by side in the free dim.
    XW = XB * BL
    xq = nc.dram_tensor("xq", [tb, 120, XW], F16, kind="ExternalInput").ap()
    # w1 fp16 triple-K stack [wh; wh; wl]
    w1trip = nc.dram_tensor("w1trip", [120, H], F16, kind="ExternalInput").ap()
    # w2 parity packs: parity p's stationary operand is w2p[p][:, :10*(p+1)]
    # with [-0.5*w2h | -0.5*w2l] in cols 10p..10p+9 and zeros below.
    w2p0 = nc.dram_tensor("w2p0", [H, 30], F16, kind="ExternalInput").ap()
    w2p1 = nc.dram_tensor("w2p1", [H, 20], F16, kind="ExternalInput").ap()
    w2p2 = nc.dram_tensor("w2p2", [H, 30], F16, kind="ExternalInput").ap()
    bias1 = nc.dram_tensor("bias1", [H, 1], F32, kind="ExternalInput").ap()
    # out[k, g, r, f]: stage k, col-group g, row r = 10p + c (parity p,
    # class-part c: 0-4 = w2h part, 5-9 = w2l part), f = 256*blk + j.
    # Step t = SPT*k + NPAR*blk + p, batch = 256*g + j.  Host decodes.
    out = nc.dram_tensor(
        "out", [nstage, NG, 30, NBLK * BG], F32, kind="ExternalOutput"
    ).ap()

    with tile.TileContext(nc) as tc:
        with (
            tc.tile_pool(name="const", bufs=1) as cpool,
            tc.tile_pool(name="state", bufs=1) as spool,
            tc.tile_pool(name="xin", bufs=8) as xpool,
            tc.tile_pool(name="sgn", bufs=6) as gpool,
            tc.tile_pool(name="stage", bufs=3) as stpool,
            tc.tile_pool(name="ps1", bufs=2, space="PSUM") as p1pool,
            tc.tile_pool(name="ps2", bufs=2, space="PSUM") as p2pool,
        ):
            w1t_s = cpool.tile([120, H], F16)
            w2p0_s = cpool.tile([H, 30], F16)
            w2p1_s = cpool.tile([H, 20], F16)
            w2p2_s = cpool.tile([H, 30], F16)
            b1_s = cpool.tile([H, 1], F32)
            nc.sync.dma_start(w1t_s[:], w1trip[:])
            nc.sync.dma_start(w2p0_s[:], w2p0[:])
            nc.sync.dma_start(w2p1_s[:], w2p1[:, :20])
            nc.sync.dma_start(w2p2_s[:], w2p2[:])
            nc.sync.dma_start(b1_s[:], bias1[:])
            w2par = [w2p0_s, w2p1_s, w2p2_s]

            m1_pool_prev = spool.tile([H, BL], F32, tag="m1a")
            nc.gpsimd.memset(m1_pool_prev[:], 0.0)
            m1_pool_alt = spool.tile([H, BL], F32, tag="m1b")
            m1_pool_alt2 = spool.tile([H, BL], F32, tag="m1c")
            m1_pool_alt3 = spool.tile([H, BL], F32, tag="m1d")
            m1_bufs = [m1_pool_alt, m1_pool_alt2, m1_pool_alt3, m1_pool_prev]
            m1_prev = m1_pool_prev
            p1_st = x_st = sg_st = None
            if probe == "no_mm1":
                p1_st = spool.tile([H, BL], F32, tag="p1s")
                nc.gpsimd.memset(p1_st[:], 0.1)
            if probe == "no_xdma":
                x_st = spool.tile([120, XW], F16, tag="xs")
                nc.sync.dma_start(x_st[:], xq[0])
            if probe == "no_act":
                sg_st = spool.tile([H, BL], F16, tag="sgs")
                nc.gpsimd.memset(sg_st[:], 1.0)

            state = {"p2": None}

            def mm2_step(tau, sg):
                """Layer-2 matmul for step tau into the staged PSUM tile.
                Runs one step behind mm1 so the PE never stalls waiting
                on the DVE->ACT chain of the same step."""
                s = tau % SPT
                p, blk = s % NPAR, s // NPAR
                if s == 0:
                    state["p2"] = p2pool.tile(
                        [H, NBLK * BG], F32, tag="p2", name="p2t"
                    )
                p2 = state["p2"]
                wz = w2par[p]
                mw = wz.shape[1]          # 30 / 20 / 30
                last = (p == NPAR - 1) or (tau == total - 1)
                for g in range(NG):
                    nc.tensor.matmul(
                        p2[32 * g : 32 * g + mw, blk * BG : (blk + 1) * BG],
                        wz[:],
                        sg[:, BG * g : BG * (g + 1)],
                        start=(p == 0), stop=last,
                        tile_position=(0, 32 * g),
                    )
                if last and (s == SPT - 1 or tau == total - 1):
                    k = (tau // SPT) % nstage
                    so = stpool.tile([H, NBLK * BG], F32, tag="so")
                    if probe != "no_copy":
                        nc.scalar.copy(so[:], p2[:])
                    if probe != "no_outdma":
                        # SWDGE (gpsimd) queue: keeps the big x-input DMAs
                        # alone on the SP HWDGE ring
                        for g in range(NG):
                            nc.gpsimd.dma_start(
                                out[k, g], so[32 * g : 32 * g + 30, :]
                            )

            sg_hist = {}
            for t in range(total):
                k, s = divmod(t, XB)
                k %= tb

                if s == 0:
                    if probe == "no_xdma":
                        xt = x_st
                    else:
                        xt = xpool.tile([120, XW], F16, tag="x")
                        nc.sync.dma_start(xt[:], xq[k])

                # mm1: cur1 = w1 @ x via one K=120 stacked pass
                # ([wh; wh; wl] . [xh; xl; xh]), split in two N=512 halves
                p1 = p1_st if probe == "no_mm1" else p1pool.tile([H, BL], F32, tag="p1")
                if probe != "no_mm1":
                    for half in (0, BH):
                        nc.tensor.matmul(
                            p1[:, half : half + BH],
                            w1t_s[:],
                            xt[:, s * BL + half : s * BL + half + BH],
                            start=True, stop=True,
                        )

                # layer-2 matmul, MM2_DELAY steps behind: by the time
                # mm2(t-d) sits in the PE FIFO its sg is already ready,
                # so it never blocks mm1(t+1) behind it (a short delay
                # puts ACT's sign latency inside the PE->DVE loop).
                if t >= MM2_DELAY and probe != "no_mm2":
                    mm2_step(t - MM2_DELAY, sg_hist.pop(t - MM2_DELAY))

                # m1 = beta*m1 - (m1 > 1) + cur1 + b1  (ping-pong buffers
                # so the next step's write doesn't WAR-wait on ACT's read)
                m1 = m1_bufs[t % 4]
                if probe != "no_dve":
                    nc.vector._custom_dve(
                        SNN_OP, out=m1[:], in0=m1_prev[:], in1=p1[:],
                        s0=b1_s[:, 0:1], s1=THR, imm2=BETA,
                    )
                m1_prev = m1

                # sg = sign(1 - m1)  (= -sign(m1-1); spk1 = (1 - sg)/2)
                if probe == "no_act":
                    sg = sg_st
                else:
                    sg = gpool.tile([H, BL], F16, tag="sg")
                    nc.scalar.activation(
                        sg[:], m1[:], mybir.ActivationFunctionType.Sign,
                        bias=1.0, scale=-1.0,
                    )
                sg_hist[t] = sg

            if probe != "no_mm2":
                for tau in range(max(0, total - MM2_DELAY), total):
                    mm2_step(tau, sg_hist.pop(tau))

    nc.compile()
    return nc


_MODULE_CACHE: dict = {}


def _get_module(t_steps: int = T):
    if t_steps not in _MODULE_CACHE:
        _MODULE_CACHE[t_steps] = build_module(t_steps)
    return _MODULE_CACHE[t_steps]


# --------------------------------------------------------------------------
# Host-side sharding / gather
# --------------------------------------------------------------------------

def _fp16_pair(a):
    hi = a.astype(np.float16)
    lo = (a - hi.astype(np.float32)).astype(np.float16)
    return hi, lo


def make_in_maps(x, w1, b1, w2, b2, t_steps: int = T):
    x = np.asarray(x, dtype=np.float32)
    w1 = np.asarray(w1, dtype=np.float32)
    b1 = np.asarray(b1, dtype=np.float32)
    w2 = np.asarray(w2, dtype=np.float32)
    tb = t_steps // XB

    w1h, w1l = _fp16_pair(w1.T)                           # [F, H] each
    w1trip = np.zeros((120, H), np.float16)
    w1trip[0:F] = w1h
    w1trip[F : 2 * F] = w1h
    w1trip[2 * F : 3 * F] = w1l

    w2nh, w2nl = _fp16_pair((-0.5 * w2).T)                # [H, C]
    w2p = []
    for p in range(NPAR):
        wz = np.zeros((H, 30 if p != 1 else 20), np.float16)
        wz[:, 10 * p : 10 * p + C] = w2nh
        wz[:, 10 * p + C : 10 * p + 2 * C] = w2nl
        w2p.append(wz)

    bias1 = np.ascontiguousarray(b1[:, None])

    in_maps = []
    for c in range(NCORES):
        xc = x[c * BL : (c + 1) * BL, :t_steps, :]        # [BL, t, F]
        xt_ = xc.transpose(1, 2, 0)                       # [t, F, BL]
        xh16, xl16 = _fp16_pair(xt_)
        trip = np.concatenate([xh16, xl16, xh16], axis=1)  # [t, 120, BL]
        xqc = (
            trip.reshape(tb, XB, 120, BL)
            .transpose(0, 2, 1, 3)
            .reshape(tb, 120, XB * BL)
        )
        in_maps.append(
            {
                "xq": np.ascontiguousarray(xqc),
                "w1trip": w1trip,
                "w2p0": w2p[0],
                "w2p1": w2p[1],
                "w2p2": w2p[2],
                "bias1": bias1,
            }
        )
    return in_maps


def postprocess(results, w2, b2, t_steps: int = T):
    """results: per-core dicts with 'out' [nstage, NG, 30, 1024] raw cur2
    partials (w2h and w2l rows).  Host combines, scans m2, thresholds."""
    w2 = np.asarray(w2, dtype=np.float32)
    b2 = np.asarray(b2, dtype=np.float32)
    w2nh, w2nl = _fp16_pair((-0.5 * w2).T)
    w_eff = w2nh.astype(np.float32) + w2nl.astype(np.float32)
    corr = (-w_eff.sum(axis=0) + b2).astype(np.float32)   # [C]

    nstage = (t_steps + SPT - 1) // SPT
    cur2 = np.empty((t_steps, C, B), np.float32)
    for c in range(NCORES):
        r = results[c]["out"]                             # [k, g, 30, 1024]
        a = r.reshape(nstage, NG, NPAR, 2 * C, NBLK, BG)
        a = a.transpose(0, 4, 2, 3, 1, 5)                 # k, blk, p, c2, g, j
        a = a.reshape(nstage * NBLK * NPAR, 2 * C, NG * BG)[:t_steps]
        cur2[:, :, c * BL : (c + 1) * BL] = a[:, :C] + a[:, C:]
    cur2 += corr[None, :, None]

    m2 = np.zeros((C, B), np.float32)
    spk = np.empty((t_steps, B, C), np.float32)
    beta = np.float32(BETA)
    for t in range(t_steps):
        reset = (m2 > THR).astype(np.float32)
        m2 = beta * m2 + cur2[t] - reset
        spk[t] = (m2 > THR).T
    return spk


def kernel(x, w1, b1, w2, b2):
    nc = _get_module(T)
    in_maps = make_in_maps(x, w1, b1, w2, b2, T)
    res = run_bass_kernel_spmd(nc, in_maps, core_ids=list(range(NCORES)))
    return postprocess(res.results, w2, b2, T)


# revision 19
# speedup vs baseline: 1.2848x; 1.2848x over previous
"""Trainium2 Bass kernel for nn_AudioSNN: 2-layer spiking NN (snntorch Leaky).

Reference semantics per timestep t (over T=200 steps):
    cur1 = x_t @ w1.T + b1                      # [B, 128]
    m1   = 0.9*m1 + cur1 - (m1_prev > 1)        # reset-by-subtract
    spk1 = (m1 > 1)
    cur2 = spk1 @ w2.T + b2                     # [B, 5]
    m2   = 0.9*m2 + cur2 - (m2_prev > 1)
    out[t] = spk2 = (m2 > 1)

Strategy (pure data-parallel over batch, 8 cores x 1024 batch rows):
  - Transposed layout: states kept as [feature, batch] so H=128 sits on
    SBUF partitions and batch on the free dim.
  - Engine budget per step per core (the design is balanced around the
    DVE, which is the irreducible bottleneck -- the m1 recurrence is a
    2-source fp32 elementwise op, which only the DVE can run, at 1x):
      DVE : one fused membrane update m1 = m1*beta - (m1>1) + cur1 + b1
            on [128, 1024] fp32, ~1.19 us + ~0.1 us issue gap.
      ACT : sg = sign(1 - m1) -> fp16 (~1.04 us), plus an amortized
            full-tile PSUM->SBUF copy of staged layer-2 outputs
            (~1.0 us / 12 steps = 83 ns/step).
      PE  : mm1 (K=120 stacked fp16 hi/lo pass, 2x N=512) and a single
            merged mm2 stream (w2 hi and lo packed side-by-side in the
            stationary operand's M columns, 4 col-tile groups x N=256),
            ~0.88 us total.  mm2 trails mm1 by MM2_DELAY=3 steps so the
            DVE->ACT sign latency never sits in front of the next mm1
            in the PE FIFO (1-step delay costs ~260 ns/step).
      DMA : x in (245 KB/step) on the SP HWDGE queue; staged cur2 out
            (43 KB/step) on the gpsimd SWDGE queue so the two never
            serialize behind each other.
  - Layer 2's membrane recurrence runs on the HOST: the device ships the
    raw per-step cur2 partial sums (hi and lo rows) and the host adds
    them, applies the bias correction, scans m2 and thresholds.  This
    removes the [128, 256] m2 DVE op (~0.39 us/step) from the bottleneck
    engine.
  - cur2 staging packs THREE timesteps into each [32-row x 256-col]
    PSUM region: parity p lands in partition rows 32g+10p..10p+9.
    Parity 0 uses an M=30 stationary tile (real weights in cols 0-9,
    zeros in 10-29) with start=True so all 30 rows get has_written set;
    parities 1-2 accumulate onto the zeros.  12 steps fill a
    [128, 1024] tile (2 PSUM banks), then one ScalarE copy moves it to
    SBUF and ONE full-width [128, 1024] DMA ships it (a 30-partition
    DMA would run at ~30/128 of peak; the padded full tile is ~4x
    cheaper in queue time).
  - All matmuls run in fp16 with hi/lo split pairs (x = xh + xl exactly
    to ~2^-22 rel; w likewise), accumulated exactly in fp32 PSUM:
    mm1 = wh@xh + wh@xl + wl@xh (one K=120 stacked pass); mm2 streams
    sg = -sign(m1-1) once against [-0.5*w2h | -0.5*w2l] columns.
  - Measured (min over enqueue-pipeline slopes of a 17x-repeat module):
    ~1.25 us/step steady state, ~260-280 us/exec device time; scheduler
    cost-model sim agrees (270.6 us span, DVE 88% busy).
"""

import numpy as np

import concourse.bacc as bacc
import concourse.mybir as mybir
import concourse.tile as tile
import concourse.dve_ops as dve_ops
from concourse.dve_ops import DveOp
from concourse.dve_spec import Spec, Src0, Src1, C0, C1, C2, lower as dve_lower
from concourse.dve_uop import DveOpSpec
from concourse.bass_utils import run_bass_kernel_spmd

F32 = mybir.dt.float32
F16 = mybir.dt.float16

B, T, F, H, C = 8192, 200, 40, 128, 5
NCORES = 8
BL = B // NCORES          # 1024 batch rows per core
BH = BL // 2              # 512 per mm1 matmul (PSUM bank limit)
BETA, THR = 0.9, 1.0
NG = 4                    # col-tile groups for layer 2 (tile_position)
BG = BL // NG             # 256 batch cols per group
XB = 4                    # timesteps per x DMA batch
MM2_DELAY = 3             # steps mm2 trails mm1 in the PE queue
NPAR = 3                  # timesteps packed per PSUM region (partition rows)
NBLK = 4                  # 256-col blocks per [128, 1024] stage tile
SPT = NPAR * NBLK         # 12 timesteps per staged output tile


# --------------------------------------------------------------------------
# Custom DVE op: fused SNN membrane update
# --------------------------------------------------------------------------

def _snn_ref(in0, in1, s0, s1, imm2):
    out = (
        in0.astype(np.float32) * imm2
        - (in0 > s1).astype(np.float32)
        + in1.astype(np.float32)
        + s0
    )
    return out.astype(np.float32)


def _register_snn_op() -> DveOp:
    """out = in0*imm2 - (in0 > s1) + in1 + s0"""
    name = "SNN_MEMBRANE_STEP"
    for op in dve_ops.OPS:
        if op.name == name:
            return op
    body = Src0 * C2 - (Src0 > C1) + Src1 + C0
    spec = Spec(body=body, reference=_snn_ref)
    shas = {}
    for ver in ("v3", "v4"):
        uops = dve_lower(spec, ver=ver)
        shas[ver] = DveOpSpec(name=name, opcode=0, uops=uops, rd1_en=True).sha(ver)
    op = DveOp(name, spec, subdim=False, uops_sha=shas)
    dve_ops.OPS.append(op)
    dve_ops._SUB_OPCODE_FOR_NAME[op.name] = (
        dve_ops._CUSTOM_DVE_ROW_BASE + len(dve_ops.OPS) - 1
    )
    dve_ops.CUSTOM_DVE_SPECS[op.name] = spec
    return op


SNN_OP = _register_snn_op()


# --------------------------------------------------------------------------
# Bass module
# --------------------------------------------------------------------------

def build_module(t_steps: int = T, probe: str = "", repeats: int = 1):
    """repeats > 1 loops the whole computation back-to-back inside the
    module (state carries over, outputs rewritten) — timing-only builds
    that amortize the per-dispatch overhead."""
    assert t_steps % XB == 0
    tb = t_steps // XB
    nstage = (t_steps + SPT - 1) // SPT
    total = repeats * t_steps
    nc = bacc.Bacc("TRN2", target_bir_lowering=False, debug=False)

    # x packed for the K-stacked 3-pass mm1: rows 0-39 = xh, rows 40-79
    # = xl, rows 80-119 = xh again (pairs with [wh; wh; wl] on the weight
    # side).  XB steps side by side in the free dim.
    XW = XB * BL
    xq = nc.dram_tensor("xq", [tb, 120, XW], F16, kind="ExternalInput").ap()
    # w1 fp16 triple-K stack [wh; wh; wl]
    w1trip = nc.dram_tensor("w1trip", [120, H], F16, kind="ExternalInput").ap()
    # w2 parity packs: parity p's stationary operand is w2p[p][:, :10*(p+1)]
    # with [-0.5*w2h | -0.5*w2l] in cols 10p..10p+9 and zeros below.
    w2p0 = nc.dram_tensor("w2p0", [H, 30], F16, kind="ExternalInput").ap()
    w2p1 = nc.dram_tensor("w2p1", [H, 20], F16, kind="ExternalInput").ap()
    w2p2 = nc.dram_tensor("w2p2", [H, 30], F16, kind="ExternalInput").ap()
    bias1 = nc.dram_tensor("bias1", [H, 1], F32, kind="ExternalInput").ap()
    # out[k, 32g + r, f]: stage k, col-group g, row r = 10p + c (parity p,
    # class-part c: 0-4 = w2h part, 5-9 = w2l part; r >= 30 garbage),
    # f = 256*blk + j.  Step t = SPT*k + NPAR*blk + p, batch = 256*g + j.
    # Shipped as one full-width [128, 1024] DMA per stage: a 30-partition
    # DMA runs at ~30/128 of peak, so the padded full-tile DMA is ~4x
    # faster than 4 sliced ones.  Host decodes and drops the pad rows.
    out = nc.dram_tensor(
        "out", [nstage, H, NBLK * BG], F32, kind="ExternalOutput"
    ).ap()

    with tile.TileContext(nc) as tc:
        with (
            tc.tile_pool(name="const", bufs=1) as cpool,
            tc.tile_pool(name="state", bufs=1) as spool,
            tc.tile_pool(name="xin", bufs=8) as xpool,
            tc.tile_pool(name="sgn", bufs=6) as gpool,
            tc.tile_pool(name="stage", bufs=3) as stpool,
            tc.tile_pool(name="ps1", bufs=2, space="PSUM") as p1pool,
            tc.tile_pool(name="ps2", bufs=2, space="PSUM") as p2pool,
        ):
            w1t_s = cpool.tile([120, H], F16)
            w2p0_s = cpool.tile([H, 30], F16)
            w2p1_s = cpool.tile([H, 20], F16)
            w2p2_s = cpool.tile([H, 30], F16)
            b1_s = cpool.tile([H, 1], F32)
            nc.sync.dma_start(w1t_s[:], w1trip[:])
            nc.sync.dma_start(b1_s[:], bias1[:])
            # w2 packs aren't needed until mm2(0) fires (MM2_DELAY steps
            # in) — issue them behind the first x chunk
            w2par = [w2p0_s, w2p1_s, w2p2_s]

            m1_pool_prev = spool.tile([H, BL], F32, tag="m1a")
            nc.gpsimd.memset(m1_pool_prev[:], 0.0)
            m1_pool_alt = spool.tile([H, BL], F32, tag="m1b")
            m1_pool_alt2 = spool.tile([H, BL], F32, tag="m1c")
            m1_pool_alt3 = spool.tile([H, BL], F32, tag="m1d")
            m1_bufs = [m1_pool_alt, m1_pool_alt2, m1_pool_alt3, m1_pool_prev]
            m1_prev = m1_pool_prev
            p1_st = x_st = sg_st = None
            if probe == "no_mm1":
                p1_st = spool.tile([H, BL], F32, tag="p1s")
                nc.gpsimd.memset(p1_st[:], 0.1)
            if probe == "no_xdma":
                x_st = spool.tile([120, XW], F16, tag="xs")
                nc.sync.dma_start(x_st[:], xq[0])
            if probe == "no_act":
                sg_st = spool.tile([H, BL], F16, tag="sgs")
                nc.gpsimd.memset(sg_st[:], 1.0)

            state = {"p2": None}

            def mm2_step(tau, sg):
                """Layer-2 matmul for step tau into the staged PSUM tile.
                Runs one step behind mm1 so the PE never stalls waiting
                on the DVE->ACT chain of the same step."""
                s = tau % SPT
                p, blk = s % NPAR, s // NPAR
                if s == 0:
                    state["p2"] = p2pool.tile(
                        [H, NBLK * BG], F32, tag="p2", name="p2t"
                    )
                p2 = state["p2"]
                wz = w2par[p]
                mw = wz.shape[1]          # 30 / 20 / 30
                last = (p == NPAR - 1) or (tau == total - 1)
                for g in range(NG):
                    nc.tensor.matmul(
                        p2[32 * g : 32 * g + mw, blk * BG : (blk + 1) * BG],
                        wz[:],
                        sg[:, BG * g : BG * (g + 1)],
                        start=(p == 0), stop=last,
                        tile_position=(0, 32 * g),
                    )
                if last and (s == SPT - 1 or tau == total - 1):
                    k = (tau // SPT) % nstage
                    so = stpool.tile([H, NBLK * BG], F32, tag="so")
                    if probe != "no_copy":
                        nc.scalar.copy(so[:], p2[:])
                    if probe != "no_outdma":
                        # SWDGE (gpsimd) queue: keeps the big x-input DMAs
                        # alone on the SP HWDGE ring
                        nc.gpsimd.dma_start(out[k], so[:])

            sg_hist = {}
            for t in range(total):
                k, s = divmod(t, XB)
                k %= tb

                if s == 0:
                    if probe == "no_xdma":
                        xt = x_st
                    else:
                        xt = xpool.tile([120, XW], F16, tag="x")
                        if t == 0:
                            # quarter-DMAs so mm1(0) starts after ~790 ns
                            # of x transfer instead of the full 3.2 us
                            for q in range(XB):
                                nc.sync.dma_start(
                                    xt[:, q * BL : (q + 1) * BL],
                                    xq[k][:, q * BL : (q + 1) * BL],
                                )
                            nc.sync.dma_start(w2p0_s[:], w2p0[:])
                            nc.sync.dma_start(w2p1_s[:], w2p1[:, :20])
                            nc.sync.dma_start(w2p2_s[:], w2p2[:])
                        else:
                            nc.sync.dma_start(xt[:], xq[k])

                # mm1: cur1 = w1 @ x via one K=120 stacked pass
                # ([wh; wh; wl] . [xh; xl; xh]), split in two N=512 halves
                p1 = p1_st if probe == "no_mm1" else p1pool.tile([H, BL], F32, tag="p1")
                if probe != "no_mm1":
                    for half in (0, BH):
                        nc.tensor.matmul(
                            p1[:, half : half + BH],
                            w1t_s[:],
                            xt[:, s * BL + half : s * BL + half + BH],
                            start=True, stop=True,
                        )

                # layer-2 matmul, MM2_DELAY steps behind: by the time
                # mm2(t-d) sits in the PE FIFO its sg is already ready,
                # so it never blocks mm1(t+1) behind it (a short delay
                # puts ACT's sign latency inside the PE->DVE loop).
                if t >= MM2_DELAY and probe != "no_mm2":
                    mm2_step(t - MM2_DELAY, sg_hist.pop(t - MM2_DELAY))

                # m1 = beta*m1 - (m1 > 1) + cur1 + b1  (ping-pong buffers
                # so the next step's write doesn't WAR-wait on ACT's read)
                m1 = m1_bufs[t % 4]
                if probe != "no_dve":
                    nc.vector._custom_dve(
                        SNN_OP, out=m1[:], in0=m1_prev[:], in1=p1[:],
                        s0=b1_s[:, 0:1], s1=THR, imm2=BETA,
                    )
                m1_prev = m1

                # sg = sign(1 - m1)  (= -sign(m1-1); spk1 = (1 - sg)/2)
                if probe == "no_act":
                    sg = sg_st
                else:
                    sg = gpool.tile([H, BL], F16, tag="sg")
                    nc.scalar.activation(
                        sg[:], m1[:], mybir.ActivationFunctionType.Sign,
                        bias=1.0, scale=-1.0,
                    )
                sg_hist[t] = sg

            if probe != "no_mm2":
                for tau in range(max(0, total - MM2_DELAY), total):
                    mm2_step(tau, sg_hist.pop(tau))

    nc.compile()
    return nc


_MODULE_CACHE: dict = {}


def _get_module(t_steps: int = T):
    if t_steps not in _MODULE_CACHE:
        _MODULE_CACHE[t_steps] = build_module(t_steps)
    return _MODULE_CACHE[t_steps]


# --------------------------------------------------------------------------
# Host-side sharding / gather
# --------------------------------------------------------------------------

def _fp16_pair(a):
    hi = a.astype(np.float16)
    lo = (a - hi.astype(np.float32)).astype(np.float16)
    return hi, lo


def make_in_maps(x, w1, b1, w2, b2, t_steps: int = T):
    x = np.asarray(x, dtype=np.float32)
    w1 = np.asarray(w1, dtype=np.float32)
    b1 = np.asarray(b1, dtype=np.float32)
    w2 = np.asarray(w2, dtype=np.float32)
    tb = t_steps // XB

    w1h, w1l = _fp16_pair(w1.T)                           # [F, H] each
    w1trip = np.zeros((120, H), np.float16)
    w1trip[0:F] = w1h
    w1trip[F : 2 * F] = w1h
    w1trip[2 * F : 3 * F] = w1l

    w2nh, w2nl = _fp16_pair((-0.5 * w2).T)                # [H, C]
    w2p = []
    for p in range(NPAR):
        wz = np.zeros((H, 30 if p != 1 else 20), np.float16)
        wz[:, 10 * p : 10 * p + C] = w2nh
        wz[:, 10 * p + C : 10 * p + 2 * C] = w2nl
        w2p.append(wz)

    bias1 = np.ascontiguousarray(b1[:, None])

    in_maps = []
    for c in range(NCORES):
        xc = x[c * BL : (c + 1) * BL, :t_steps, :]        # [BL, t, F]
        xt_ = xc.transpose(1, 2, 0)                       # [t, F, BL]
        xh16, xl16 = _fp16_pair(xt_)
        trip = np.concatenate([xh16, xl16, xh16], axis=1)  # [t, 120, BL]
        xqc = (
            trip.reshape(tb, XB, 120, BL)
            .transpose(0, 2, 1, 3)
            .reshape(tb, 120, XB * BL)
        )
        in_maps.append(
            {
                "xq": np.ascontiguousarray(xqc),
                "w1trip": w1trip,
                "w2p0": w2p[0],
                "w2p1": w2p[1],
                "w2p2": w2p[2],
                "bias1": bias1,
            }
        )
    return in_maps


def postprocess(results, w2, b2, t_steps: int = T):
    """results: per-core dicts with 'out' [nstage, NG, 30, 1024] raw cur2
    partials (w2h and w2l rows).  Host combines, scans m2, thresholds."""
    w2 = np.asarray(w2, dtype=np.float32)
    b2 = np.asarray(b2, dtype=np.float32)
    w2nh, w2nl = _fp16_pair((-0.5 * w2).T)
    w_eff = w2nh.astype(np.float32) + w2nl.astype(np.float32)
    corr = (-w_eff.sum(axis=0) + b2).astype(np.float32)   # [C]

    nstage = (t_steps + SPT - 1) // SPT
    cur2 = np.empty((t_steps, C, B), np.float32)
    for c in range(NCORES):
        r = results[c]["out"]                             # [k, 128, 1024]
        a = r.reshape(nstage, NG, 32, NBLK * BG)[:, :, :30]
        a = a.reshape(nstage, NG, NPAR, 2 * C, NBLK, BG)
        a = a.transpose(0, 4, 2, 3, 1, 5)                 # k, blk, p, c2, g, j
        a = a.reshape(nstage * NBLK * NPAR, 2 * C, NG * BG)[:t_steps]
        cur2[:, :, c * BL : (c + 1) * BL] = a[:, :C] + a[:, C:]
    cur2 += corr[None, :, None]

    m2 = np.zeros((C, B), np.float32)
    spk = np.empty((t_steps, B, C), np.float32)
    beta = np.float32(BETA)
    for t in range(t_steps):
        reset = (m2 > THR).astype(np.float32)
        m2 = beta * m2 + cur2[t] - reset
        spk[t] = (m2 > THR).T
    return spk


def kernel(x, w1, b1, w2, b2):
    nc = _get_module(T)
    in_maps = make_in_maps(x, w1, b1, w2, b2, T)
    res = run_bass_kernel_spmd(nc, in_maps, core_ids=list(range(NCORES)))
    return postprocess(res.results, w2, b2, T)
